# revision 22
# baseline (speedup 1.0000x reference)
"""Trainium2 Bass kernel for: out = conv3x3(x, weight*A_w) * sigmoid(conv3x3(relu(conv3x3(x, se_w1)), se_w2))

Sharding: data-parallel over batch B=8 -> 8 NeuronCores (one image per core);
weights replicated. A_w is folded into the conv weight on the host (f32
multiply, then bf16 cast), so the device sees one fused weight tensor.

Per-core kernel (direct conv as implicit GEMM on the TensorEngine):
  - x stored column-padded [ci, 56, 58] bf16 in SBUF (pad cols zeroed,
    +1-element guards at both flat ends) so every 3x3 tap is a contiguous
    1-D shifted window (the matmul ISA requires single-free-dim operands).
  - row taps at the image top/bottom use clipped row ranges; the center tap
    is issued first per ci-block pass (full coverage, start=True), the
    clipped taps accumulate -> exact zero-padding semantics.
  - compute dtype bf16 (fp32 PSUM accumulate), rel-err vs fp32 ~3e-3.
  - x DMA is row-chunked (4 chunks) and ordered ahead of the big main-conv
    weight so the SE branch starts matmuls at ~10us instead of waiting for
    the full 2.9MB input load.
  - schedule: the SE chain (pack -> strip-drain -> selector -> relu/sigmoid)
    is software-pipelined with sel lagging pack, and dense 18-matmul main
    groups are interleaved one-per-SE-step (from step 10, pausing during
    the SE tail) so the PE rides through every ~0.6us PSUM-drain latency;
    strip drains are split vector/scalar to balance the two engines.
  - every main group is fused: attention multiply straight out of PSUM and
    per-tile output DMA, so the 3.3MB output streams during compute and the
    kernel ends ~1.5us after the last matmul.
  - ScalarE activation tables (identity/relu/sigmoid) are pre-warmed with
    dummy activations during the DMA wait.

Rejected variants (measured slower): SE-branch-first scheduling (exposes
the serial PSUM-drain chain, 91us); 1-D F(2,3) Winograd main conv (24 MMs
of free 392 per 14-row tile instead of 36 of 464 per 16 rows, but the
7 extra [128,392] elementwise ops per group saturate Vector/Scalar, 101us).
"""

import numpy as np

import concourse.bass as bass  # noqa: F401
import concourse.mybir as mybir
import concourse.tile as tile
from concourse import bacc
from concourse.bass_utils import run_bass_kernel_spmd
from concourse.masks import make_identity

B, C, H, W = 8, 256, 56, 56
HW = H * W
WP = W + 2                      # padded row width (c=0 left pad, c=57 right pad)
HWP = H * WP                    # 3248
CMID = 16
N_CORES = 8
RT = 8                          # output rows per PSUM tile
NT = H // RT                    # 7
F32 = mybir.dt.float32
BF16 = mybir.dt.bfloat16

# x DMA row-chunk boundaries: chunk A rows [0,10) serves SE tile 0,
# B rows [10,18) tile 1, C rows [18,34) tiles 2-3, D rows [34,56) tiles 4-6.
CH_A = 10
CH_B = 18
CH_C = 34

# center tap first within each ci-block pass
TAPS = [(0, 0)] + [
    (dh, dw) for dh in (-1, 0, 1) for dw in (-1, 0, 1) if (dh, dw) != (0, 0)
]


def _rows(r0, dh):
    """Clipped local row range [rl, rh) of a tile at base row r0 for row-tap dh."""
    return max(0, -dh - r0), min(RT, H - dh - r0)


def build():
    nc = bacc.Bacc("TRN2", target_bir_lowering=False, debug=False, num_devices=N_CORES)

    # x pre-padded on host: [ci-block, 128, 1 + 56*58 + 1] bf16, zero pad
    # columns and flat-end guards baked in
    x_d = nc.dram_tensor("xpad", [2, 128, HWP + 2], BF16, kind="ExternalInput").ap()
    # (weight * A_w) transposed on host: [ci, kh, kw, co] -> [2, 128, 9*256]
    wm_d = nc.dram_tensor("wmodT", [2, 128, 9 * 256], BF16, kind="ExternalInput").ap()
    # SE weights pre-packed on host: kw groups at 32-col strides per kh
    w1p_d = nc.dram_tensor("se_w1P", [2, 128, 3 * 96], BF16, kind="ExternalInput").ap()
    w2p_d = nc.dram_tensor("se_w2P", [CMID, 3 * 96], BF16, kind="ExternalInput").ap()
    # output in padded layout [ci-block, 128, 56*58]; host strips pad cols
    out_d = nc.dram_tensor("outp", [2, 128, HWP], F32, kind="ExternalOutput").ap()

    with tile.TileContext(nc) as tc:
        with (
            tc.tile_pool(name="sb", bufs=1) as sb,
            tc.tile_pool(name="ps", space="PSUM", bufs=2) as ps,
        ):
            asb = sb.tile([128, HWP], F32, name="asb")
            osb = [sb.tile([128, HWP], F32, name=f"osb{c}") for c in range(2)]
            # +2: one guard element at each flat end (dw=+-1 at image corners)
            xs = [sb.tile([128, HWP + 2], BF16, name=f"xs{i}") for i in range(2)]
            wmod = [sb.tile([128, 9 * 256], BF16, name=f"wmod{i}") for i in range(2)]
            mid = sb.tile([CMID, HWP + 2], BF16, name="mid")
            identE = sb.tile([96, CMID], BF16, name="identE")
            identTE = sb.tile([96, 128], BF16, name="identTE")
            u1pp = [sb.tile([96, RT * WP], BF16, name=f"u1pp{k}") for k in range(2)]
            u2pp = [sb.tile([96, RT * WP], BF16, name=f"u2pp{k}") for k in range(2)]
            w2pack = sb.tile([CMID, 3 * 96], BF16, name="w2pack")
            w1pack = [sb.tile([128, 3 * 96], BF16, name=f"w1pack{i}") for i in range(2)]

            # -------- loads --------
            # SE weights + first x row-chunk first (the PE's first deps);
            # remaining x chunks next; the big main-conv weight streams last
            # on the scalar queue (not needed until the main phase).
            # All x chunks stream on the Sync queue, blocks interleaved, so
            # both ci-blocks of a chunk land together; the big main-conv
            # weight is interleaved after chunk B (needed once the first
            # main groups are schedulable, ~16us). SE weights ride GpSimd;
            # the Scalar queue stays free of input DMAs (it runs
            # activations).
            fA = 1 + CH_A * WP
            fB = 1 + CH_B * WP
            fC = 1 + CH_C * WP
            nc.gpsimd.dma_start(w1pack[0], w1p_d[0])
            nc.gpsimd.dma_start(w1pack[1], w1p_d[1])
            nc.gpsimd.dma_start(w2pack, w2p_d)
            for a, b in ((0, fA), (fA, fB)):
                nc.sync.dma_start(xs[0][:, a:b], x_d[0][:, a:b])
                nc.sync.dma_start(xs[1][:, a:b], x_d[1][:, a:b])
            nc.sync.dma_start(wmod[0], wm_d[0])
            for a, b in ((fB, fC), (fC, HWP + 2)):
                nc.sync.dma_start(xs[0][:, a:b], x_d[0][:, a:b])
                nc.sync.dma_start(xs[1][:, a:b], x_d[1][:, a:b])
            nc.sync.dma_start(wmod[1], wm_d[1])

            def pad_memset(tl, np_):
                nc.vector.memset(tl[:np_, 0:2], 0.0)
                nc.vector.memset(tl[:np_, HWP : HWP + 2], 0.0)
                pads = tl[:np_, 1 + W + 1 : 1 + W + 1 + (H - 1) * WP].rearrange(
                    "p (h c) -> p h c", c=WP
                )
                nc.vector.memset(pads[:, :, 0:2], 0.0)

            # -------- prep (VectorE only, no PE) --------
            # pre-warm ScalarE activation tables (sigmoid/relu/identity)
            # during the DMA wait so the ~1.3us table loads don't stall the
            # SE dependency chain mid-kernel
            warm = sb.tile([1, 2], F32, name="warm")
            nc.vector.memset(warm, 0.0)
            for fn in (
                mybir.ActivationFunctionType.Identity,
                mybir.ActivationFunctionType.Relu,
                mybir.ActivationFunctionType.Sigmoid,
            ):
                nc.scalar.activation(warm[0:1, 0:1], warm[0:1, 1:2], fn)
            pad_memset(mid, CMID)
            for k in range(2):
                nc.vector.memset(u1pp[k], 0.0)
                nc.vector.memset(u2pp[k], 0.0)
            # identity selectors, one copy per 32-aligned strip (matmul
            # operands must share a 32-aligned partition base)
            nc.vector.memset(identE, 0.0)
            nc.vector.memset(identTE, 0.0)
            for g in range(3):
                make_identity(nc, identE[32 * g : 32 * g + CMID, :], nomemset=True)
                nc.vector.tensor_copy(
                    identTE[32 * g : 32 * g + CMID, :].rearrange(
                        "p (r c) -> p r c", c=CMID
                    ),
                    identE[32 * g : 32 * g + CMID, :]
                    .unsqueeze(1)
                    .broadcast_to([CMID, 8, CMID]),
                )

            mid_v = mid[:, 1 : 1 + HWP].rearrange("p (h c) -> p h c", c=WP)
            TFv = RT * WP
            wmod_v = [wmod[i].rearrange("p (k co) -> p k co", co=256) for i in range(2)]

            # -------- conv group emitters --------
            # SE convs: the 3 kw taps are packed into the stationary columns
            # (48 = 3 kw x 16 ch), then reduced across partition groups with
            # +-1-shifted identity matmuls. Junk in pad columns only.
            def conv1_pack(t):
                r0 = t * RT
                mps = ps.tile([96, TFv], F32, name="mps96", tag="pack", bufs=3)
                n_mm = 0
                for i in range(2):
                    for dh in (0, -1, 1):
                        kh = dh + 1
                        rl, rh = _rows(r0, dh)
                        n_mm += 1
                        nc.tensor.matmul(
                            mps[:, rl * WP : rh * WP],
                            w1pack[i][:, kh * 96 : (kh + 1) * 96],
                            xs[i][:, 1 + (r0 + rl + dh) * WP :][:128, : (rh - rl) * WP],
                            start=(n_mm == 1),
                            stop=(n_mm == 6),
                        )
                u = u1pp[t % 2]
                # drain each kw strip with its +-1 column shift baked in, so
                # one K=96 selector matmul can reduce without further shifts.
                # Strips split vector/scalar/vector to keep ScalarE (which
                # also runs relu+sigmoid) off the critical path.
                ident = mybir.ActivationFunctionType.Identity
                nc.vector.tensor_copy(u[0:16, 1:TFv], mps[0:16, 0 : TFv - 1])
                nc.scalar.activation(u[32:48, :], mps[32:48, :], ident)
                nc.vector.tensor_copy(u[64:80, 0 : TFv - 1], mps[64:80, 1:TFv])
                return u

            def conv1_sel(t, u):
                r0 = t * RT
                mid_ps = ps.tile([CMID, TFv], F32, name="mid_ps", tag="red", bufs=2)
                nc.tensor.matmul(mid_ps, identE, u, start=True, stop=True)
                mpv = mid_ps.rearrange("p (h c) -> p h c", c=WP)
                nc.scalar.activation(
                    mid_v[:, r0 : r0 + RT, 1 : W + 1],
                    mpv[:, :, 1 : W + 1],
                    mybir.ActivationFunctionType.Relu,
                )

            def conv2_pack(t):
                r0 = t * RT
                ups = ps.tile([96, TFv], F32, name="u2ps", tag="pack", bufs=3)
                n_mm = 0
                for dh in (0, -1, 1):
                    kh = dh + 1
                    rl, rh = _rows(r0, dh)
                    n_mm += 1
                    nc.tensor.matmul(
                        ups[:, rl * WP : rh * WP],
                        w2pack[:, kh * 96 : (kh + 1) * 96],
                        mid[:, 1 + (r0 + rl + dh) * WP :][:CMID, : (rh - rl) * WP],
                        start=(n_mm == 1),
                        stop=(n_mm == 3),
                    )
                u = u2pp[t % 2]
                ident = mybir.ActivationFunctionType.Identity
                nc.vector.tensor_copy(u[0:16, 1:TFv], ups[0:16, 0 : TFv - 1])
                nc.scalar.activation(u[32:48, :], ups[32:48, :], ident)
                nc.vector.tensor_copy(u[64:80, 0 : TFv - 1], ups[64:80, 1:TFv])
                return u

            def conv2_sel(t, u):
                r0 = t * RT
                aps = ps.tile([128, TFv], F32, name="aps", tag="red", bufs=2)
                nc.tensor.matmul(aps, identTE, u, start=True, stop=True)
                nc.scalar.activation(
                    asb[:, r0 * WP : (r0 + RT) * WP],
                    aps,
                    mybir.ActivationFunctionType.Sigmoid,
                )

            def main_mms(t, c):
                r0 = t * RT
                yps = ps.tile([128, RT * WP], F32, name="yps", tag="yps", bufs=3)
                n_mm = 0
                for i in range(2):
                    for dh, dw in TAPS:
                        k = (dh + 1) * 3 + (dw + 1)
                        rl, rh = _rows(r0, dh)
                        n_mm += 1
                        nc.tensor.matmul(
                            yps[:, rl * WP : rh * WP],
                            wmod_v[i][:, k, c * 128 : (c + 1) * 128],
                            xs[i][:, 1 + (r0 + rl + dh) * WP + dw :][:128, : (rh - rl) * WP],
                            start=(n_mm == 1),
                            stop=(n_mm == 18),
                        )
                return yps

            def main_fin(t, c, yps):
                # deferred by one schedule step: by the time VectorE reaches
                # this mul, the group's matmuls are done, so SE strip drains
                # queued after it never stall behind a blocked mul
                r0 = t * RT
                dst = osb[c][:, r0 * WP : (r0 + RT) * WP]
                nc.vector.tensor_mul(dst, yps, asb[:, r0 * WP : (r0 + RT) * WP])
                q = nc.sync if (t + c) % 2 == 0 else nc.scalar
                q.dma_start(out_d[c][:, r0 * WP : (r0 + RT) * WP], dst)

            # -------- schedule ------------------------------------------
            # SE chain software-pipelined (sel lags pack by 2 tiles so the
            # ~0.6us/op PSUM drains never gate the PE); once the attention
            # tile for t is sigmoided, the two main groups for t become
            # ready and are interleaved one-per-SE-step — the dense 18-MM
            # groups keep the PE fed across every SE latency bubble and the
            # output DMA streams out during compute. Mains start at step 13
            # (~wmod + x-chunk arrival); every main is fused (no deferred
            # attention pass, no output tail).
            se_steps = [
                ("p", 0), ("p", 1), ("s", 0),
                ("p", 2), ("s", 1), ("q", 0),
                ("p", 3), ("s", 2), ("q", 1), ("r", 0),
                ("p", 4), ("s", 3), ("q", 2), ("r", 1),
                ("p", 5), ("s", 4), ("q", 3), ("r", 2),
                ("p", 6), ("s", 5), ("q", 4), ("r", 3),
                ("s", 6), ("q", 5), ("r", 4),
                ("q", 6), ("r", 5), ("r", 6),
            ]
            u1 = {}
            u2 = {}
            ready = []
            pend = None
            for idx, (kind, t) in enumerate(se_steps):
                if kind == "p":
                    u1[t] = conv1_pack(t)
                elif kind == "s":
                    conv1_sel(t, u1[t])
                elif kind == "q":
                    u2[t] = conv2_pack(t)
                else:
                    conv2_sel(t, u2[t])
                    ready.append((t, 0))
                    ready.append((t, 1))
                if 10 <= idx <= 21 and ready:
                    if pend is not None:
                        main_fin(*pend)
                    mt, mc = ready.pop(0)
                    pend = (mt, mc, main_mms(mt, mc))
            for mt, mc in ready:
                if pend is not None:
                    main_fin(*pend)
                pend = (mt, mc, main_mms(mt, mc))
            if pend is not None:
                main_fin(*pend)

    nc.compile()
    return nc


_NC = None


def make_in_maps(x, weight, A_w, se_w1, se_w2):
    import ml_dtypes

    bf16 = ml_dtypes.bfloat16
    x = np.asarray(x, dtype=np.float32)
    # pre-padded x: [B, ci-block, 128, guard + 56*58 + guard] with zero pad
    # columns (c=0, c=57) and guards
    xpad = np.zeros((B, 2, 128, HWP + 2), dtype=bf16)
    xv = xpad[:, :, :, 1 : 1 + HWP].reshape(B, 2, 128, H, WP)
    xv[:, :, :, :, 1 : W + 1] = x.reshape(B, 2, 128, H, W).astype(bf16)

    # fold A_w into the conv weight on host (f32), then transpose+cast
    wm = np.asarray(weight, dtype=np.float32) * np.asarray(A_w, dtype=np.float32)
    wmodT = np.ascontiguousarray(
        wm.transpose(1, 2, 3, 0).reshape(2, 128, 9 * 256).astype(bf16)
    )

    # SE weights pre-packed: kw groups at 32-col strides per kh slice
    w1T = np.asarray(se_w1, dtype=np.float32).transpose(1, 2, 3, 0)  # [ci,kh,kw,16]
    w1P = np.zeros((2, 128, 3, 3, 32), dtype=bf16)
    w1P[:, :, :, :, :CMID] = w1T.reshape(2, 128, 3, 3, CMID).astype(bf16)
    w1P = np.ascontiguousarray(w1P.reshape(2, 128, 3 * 96))
    w2P = np.zeros((CMID, 3, 3, 32), dtype=bf16)
    w2P[:, :, :, :CMID] = (
        np.asarray(se_w2, dtype=np.float32)[0].astype(bf16)[:, :, :, None]
    )
    w2P = np.ascontiguousarray(w2P.reshape(CMID, 3 * 96))

    in_maps = [
        {
            "xpad": np.ascontiguousarray(xpad[b]),
            "wmodT": wmodT,
            "se_w1P": w1P,
            "se_w2P": w2P,
        }
        for b in range(B)
    ]
    return in_maps


def kernel(x, weight, A_w, se_w1, se_w2):
    global _NC
    if _NC is None:
        _NC = build()
    in_maps = make_in_maps(x, weight, A_w, se_w1, se_w2)
    res = run_bass_kernel_spmd(_NC, in_maps, list(range(N_CORES)))
    out = np.stack([res.results[b]["outp"] for b in range(B)], axis=0)
    # strip pad columns: [B,2,128,56*58] -> [B,256,56,56]
    out = out.reshape(B, 2, 128, H, WP)[:, :, :, :, 1 : W + 1].reshape(B, C, H, W)
    return np.ascontiguousarray(out)


# revision 24
# speedup vs baseline: 1.1687x; 1.1687x over previous
"""Trainium2 Bass kernel for: out = conv3x3(x, weight*A_w) * sigmoid(conv3x3(relu(conv3x3(x, se_w1)), se_w2))

Sharding: data-parallel over batch B=8 -> 8 NeuronCores (one image per core);
weights replicated. A_w is folded into the conv weight on the host (f32
multiply, then bf16 cast), so the device sees one fused weight tensor.

Per-core kernel (direct conv as implicit GEMM on the TensorEngine):
  - x stored column-padded [ci, 56, 58] bf16 in SBUF (pad cols zeroed,
    +1-element guards at both flat ends) so every 3x3 tap is a contiguous
    1-D shifted window (the matmul ISA requires single-free-dim operands).
  - row taps at the image top/bottom use clipped row ranges; the center tap
    is issued first per ci-block pass (full coverage, start=True), the
    clipped taps accumulate -> exact zero-padding semantics.
  - compute dtype bf16 (fp32 PSUM accumulate), rel-err vs fp32 ~3e-3.
  - x DMA is row-chunked (4 chunks) and ordered ahead of the big main-conv
    weight so the SE branch starts matmuls at ~10us instead of waiting for
    the full 2.9MB input load; chunk C is triggered before wmod so the
    conv1 pipeline never waits on it.
  - schedule: the SE chain (pack -> strip-drain -> selector -> relu/sigmoid)
    is software-pipelined with sel lagging pack, and dense 18-matmul main
    groups are interleaved one-per-SE-step (from step 10, pausing during
    the SE tail) so the PE rides through every ~0.6us PSUM-drain latency;
    strip drains are split vector/scalar to balance the two engines.
  - every main group is fused: attention multiply straight out of PSUM and
    per-tile output DMA, so the 3.3MB output streams during compute and the
    kernel ends ~1.5us after the last matmul.
  - ScalarE activation tables (identity/relu/sigmoid) are pre-warmed with
    dummy activations during the DMA wait.

Rejected variants (measured slower): SE-branch-first scheduling (exposes
the serial PSUM-drain chain, 91us); 1-D F(2,3) Winograd main conv (24 MMs
of free 392 per 14-row tile instead of 36 of 464 per 16 rows, but the
7 extra [128,392] elementwise ops per group saturate Vector/Scalar, 101us).
"""

import numpy as np

import concourse.bass as bass  # noqa: F401
import concourse.mybir as mybir
import concourse.tile as tile
from concourse import bacc
from concourse.bass_utils import run_bass_kernel_spmd
from concourse.masks import make_identity

B, C, H, W = 8, 256, 56, 56
HW = H * W
WP = W + 2                      # padded row width (c=0 left pad, c=57 right pad)
HWP = H * WP                    # 3248
CMID = 16
N_CORES = 8
RT = 8                          # output rows per PSUM tile
NT = H // RT                    # 7
F32 = mybir.dt.float32
BF16 = mybir.dt.bfloat16

# x DMA row-chunk boundaries: chunk A rows [0,10) serves SE tile 0,
# B rows [10,18) tile 1, C rows [18,34) tiles 2-3, D rows [34,56) tiles 4-6.
CH_A = 10
CH_B = 18
CH_C = 34

# center tap first within each ci-block pass
TAPS = [(0, 0)] + [
    (dh, dw) for dh in (-1, 0, 1) for dw in (-1, 0, 1) if (dh, dw) != (0, 0)
]


def _rows(r0, dh):
    """Clipped local row range [rl, rh) of a tile at base row r0 for row-tap dh."""
    return max(0, -dh - r0), min(RT, H - dh - r0)


def build():
    nc = bacc.Bacc("TRN2", target_bir_lowering=False, debug=False, num_devices=N_CORES)

    # x pre-padded on host: [ci-block, 128, 1 + 56*58 + 1] bf16, zero pad
    # columns and flat-end guards baked in
    x_d = nc.dram_tensor("xpad", [2, 128, HWP + 2], BF16, kind="ExternalInput").ap()
    # (weight * A_w) transposed on host: [ci, kh, kw, co] -> [2, 128, 9*256]
    wm_d = nc.dram_tensor("wmodT", [2, 128, 9 * 256], BF16, kind="ExternalInput").ap()
    # SE weights pre-packed on host: kw groups at 32-col strides per kh
    w1p_d = nc.dram_tensor("se_w1P", [2, 128, 3 * 96], BF16, kind="ExternalInput").ap()
    w2p_d = nc.dram_tensor("se_w2P", [CMID, 3 * 96], BF16, kind="ExternalInput").ap()
    # output in padded layout [ci-block, 128, 56*58]; host strips pad cols
    out_d = nc.dram_tensor("outp", [2, 128, HWP], F32, kind="ExternalOutput").ap()

    with tile.TileContext(nc) as tc:
        with (
            tc.tile_pool(name="sb", bufs=1) as sb,
            tc.tile_pool(name="ps", space="PSUM", bufs=2) as ps,
        ):
            asb = sb.tile([128, HWP], F32, name="asb")
            osb = [sb.tile([128, HWP], F32, name=f"osb{c}") for c in range(2)]
            # +2: one guard element at each flat end (dw=+-1 at image corners)
            xs = [sb.tile([128, HWP + 2], BF16, name=f"xs{i}") for i in range(2)]
            wmod = [sb.tile([128, 9 * 256], BF16, name=f"wmod{i}") for i in range(2)]
            mid = sb.tile([CMID, HWP + 2], BF16, name="mid")
            identE = sb.tile([96, CMID], BF16, name="identE")
            identTE = sb.tile([96, 128], BF16, name="identTE")
            u1pp = [sb.tile([96, RT * WP], BF16, name=f"u1pp{k}") for k in range(2)]
            u2pp = [sb.tile([96, RT * WP], BF16, name=f"u2pp{k}") for k in range(2)]
            w2pack = sb.tile([CMID, 3 * 96], BF16, name="w2pack")
            w1pack = [sb.tile([128, 3 * 96], BF16, name=f"w1pack{i}") for i in range(2)]

            # -------- loads --------
            # SE weights + first x row-chunk first (the PE's first deps);
            # remaining x chunks next; the big main-conv weight streams last
            # on the scalar queue (not needed until the main phase).
            # All x chunks stream on the Sync queue, blocks interleaved, so
            # both ci-blocks of a chunk land together; the big main-conv
            # weight is interleaved after chunk B (needed once the first
            # main groups are schedulable, ~16us). SE weights ride GpSimd;
            # the Scalar queue stays free of input DMAs (it runs
            # activations).
            fA = 1 + CH_A * WP
            fB = 1 + CH_B * WP
            fC = 1 + CH_C * WP
            nc.gpsimd.dma_start(w1pack[0], w1p_d[0])
            nc.gpsimd.dma_start(w1pack[1], w1p_d[1])
            nc.gpsimd.dma_start(w2pack, w2p_d)
            for a, b in ((0, fA), (fA, fB)):
                nc.sync.dma_start(xs[0][:, a:b], x_d[0][:, a:b])
                nc.sync.dma_start(xs[1][:, a:b], x_d[1][:, a:b])
            nc.sync.dma_start(wmod[0], wm_d[0])
            for a, b in ((fB, fC), (fC, HWP + 2)):
                nc.sync.dma_start(xs[0][:, a:b], x_d[0][:, a:b])
                nc.sync.dma_start(xs[1][:, a:b], x_d[1][:, a:b])
            nc.sync.dma_start(wmod[1], wm_d[1])

            def pad_memset(tl, np_):
                nc.vector.memset(tl[:np_, 0:2], 0.0)
                nc.vector.memset(tl[:np_, HWP : HWP + 2], 0.0)
                pads = tl[:np_, 1 + W + 1 : 1 + W + 1 + (H - 1) * WP].rearrange(
                    "p (h c) -> p h c", c=WP
                )
                nc.vector.memset(pads[:, :, 0:2], 0.0)

            # -------- prep (VectorE only, no PE) --------
            # pre-warm ScalarE activation tables (sigmoid/relu/identity)
            # during the DMA wait so the ~1.3us table loads don't stall the
            # SE dependency chain mid-kernel
            warm = sb.tile([1, 2], F32, name="warm")
            nc.vector.memset(warm, 0.0)
            for fn in (
                mybir.ActivationFunctionType.Identity,
                mybir.ActivationFunctionType.Relu,
                mybir.ActivationFunctionType.Sigmoid,
            ):
                nc.scalar.activation(warm[0:1, 0:1], warm[0:1, 1:2], fn)
            pad_memset(mid, CMID)
            for k in range(2):
                nc.vector.memset(u1pp[k], 0.0)
                nc.vector.memset(u2pp[k], 0.0)
            # identity selectors, one copy per 32-aligned strip (matmul
            # operands must share a 32-aligned partition base)
            nc.vector.memset(identE, 0.0)
            nc.vector.memset(identTE, 0.0)
            for g in range(3):
                make_identity(nc, identE[32 * g : 32 * g + CMID, :], nomemset=True)
                nc.vector.tensor_copy(
                    identTE[32 * g : 32 * g + CMID, :].rearrange(
                        "p (r c) -> p r c", c=CMID
                    ),
                    identE[32 * g : 32 * g + CMID, :]
                    .unsqueeze(1)
                    .broadcast_to([CMID, 8, CMID]),
                )

            mid_v = mid[:, 1 : 1 + HWP].rearrange("p (h c) -> p h c", c=WP)
            TFv = RT * WP
            wmod_v = [wmod[i].rearrange("p (k co) -> p k co", co=256) for i in range(2)]

            # -------- conv group emitters --------
            # SE convs: the 3 kw taps are packed into the stationary columns
            # (48 = 3 kw x 16 ch), then reduced across partition groups with
            # +-1-shifted identity matmuls. Junk in pad columns only.
            def conv1_pack(t):
                r0 = t * RT
                mps = ps.tile([96, TFv], F32, name="mps96", tag="pack", bufs=3)
                n_mm = 0
                for i in range(2):
                    for dh in (0, -1, 1):
                        kh = dh + 1
                        rl, rh = _rows(r0, dh)
                        n_mm += 1
                        nc.tensor.matmul(
                            mps[:, rl * WP : rh * WP],
                            w1pack[i][:, kh * 96 : (kh + 1) * 96],
                            xs[i][:, 1 + (r0 + rl + dh) * WP :][:128, : (rh - rl) * WP],
                            start=(n_mm == 1),
                            stop=(n_mm == 6),
                        )
                u = u1pp[t % 2]
                # drain each kw strip with its +-1 column shift baked in, so
                # one K=96 selector matmul can reduce without further shifts.
                # Strips split vector/scalar/vector to keep ScalarE (which
                # also runs relu+sigmoid) off the critical path.
                ident = mybir.ActivationFunctionType.Identity
                nc.scalar.activation(u[0:16, 1:TFv], mps[0:16, 0 : TFv - 1], ident)
                nc.scalar.activation(u[32:48, :], mps[32:48, :], ident)
                nc.scalar.activation(u[64:80, 0 : TFv - 1], mps[64:80, 1:TFv], ident)
                return u

            def conv1_sel(t, u):
                r0 = t * RT
                mid_ps = ps.tile([CMID, TFv], F32, name="mid_ps", tag="red", bufs=2)
                nc.tensor.matmul(mid_ps, identE, u, start=True, stop=True)
                mpv = mid_ps.rearrange("p (h c) -> p h c", c=WP)
                nc.scalar.activation(
                    mid_v[:, r0 : r0 + RT, 1 : W + 1],
                    mpv[:, :, 1 : W + 1],
                    mybir.ActivationFunctionType.Relu,
                )

            def conv2_pack(t):
                r0 = t * RT
                ups = ps.tile([96, TFv], F32, name="u2ps", tag="pack", bufs=3)
                n_mm = 0
                for dh in (0, -1, 1):
                    kh = dh + 1
                    rl, rh = _rows(r0, dh)
                    n_mm += 1
                    nc.tensor.matmul(
                        ups[:, rl * WP : rh * WP],
                        w2pack[:, kh * 96 : (kh + 1) * 96],
                        mid[:, 1 + (r0 + rl + dh) * WP :][:CMID, : (rh - rl) * WP],
                        start=(n_mm == 1),
                        stop=(n_mm == 3),
                    )
                u = u2pp[t % 2]
                ident = mybir.ActivationFunctionType.Identity
                nc.scalar.activation(u[0:16, 1:TFv], ups[0:16, 0 : TFv - 1], ident)
                nc.scalar.activation(u[32:48, :], ups[32:48, :], ident)
                nc.scalar.activation(u[64:80, 0 : TFv - 1], ups[64:80, 1:TFv], ident)
                return u

            def conv2_sel(t, u):
                r0 = t * RT
                aps = ps.tile([128, TFv], F32, name="aps", tag="red", bufs=2)
                nc.tensor.matmul(aps, identTE, u, start=True, stop=True)
                nc.scalar.activation(
                    asb[:, r0 * WP : (r0 + RT) * WP],
                    aps,
                    mybir.ActivationFunctionType.Sigmoid,
                )

            def main_group(t, c):
                r0 = t * RT
                yps = ps.tile([128, RT * WP], F32, name="yps", tag="yps", bufs=3)
                n_mm = 0
                for i in range(2):
                    for dh, dw in TAPS:
                        k = (dh + 1) * 3 + (dw + 1)
                        rl, rh = _rows(r0, dh)
                        n_mm += 1
                        nc.tensor.matmul(
                            yps[:, rl * WP : rh * WP],
                            wmod_v[i][:, k, c * 128 : (c + 1) * 128],
                            xs[i][:, 1 + (r0 + rl + dh) * WP + dw :][:128, : (rh - rl) * WP],
                            start=(n_mm == 1),
                            stop=(n_mm == 18),
                        )
                dst = osb[c][:, r0 * WP : (r0 + RT) * WP]
                nc.vector.tensor_mul(dst, yps, asb[:, r0 * WP : (r0 + RT) * WP])
                q = nc.sync if (t + c) % 2 == 0 else nc.scalar
                q.dma_start(out_d[c][:, r0 * WP : (r0 + RT) * WP], dst)

            # -------- schedule ------------------------------------------
            # SE chain software-pipelined (sel lags pack by 2 tiles so the
            # ~0.6us/op PSUM drains never gate the PE); once the attention
            # tile for t is sigmoided, the two main groups for t become
            # ready and are interleaved one-per-SE-step — the dense 18-MM
            # groups keep the PE fed across every SE latency bubble and the
            # output DMA streams out during compute. Mains start at step 13
            # (~wmod + x-chunk arrival); every main is fused (no deferred
            # attention pass, no output tail).
            se_steps = [
                ("p", 0), ("p", 1), ("s", 0),
                ("p", 2), ("s", 1), ("q", 0),
                ("p", 3), ("s", 2), ("q", 1), ("r", 0),
                ("p", 4), ("s", 3), ("q", 2), ("r", 1),
                ("p", 5), ("s", 4), ("q", 3), ("r", 2),
                ("p", 6), ("s", 5), ("q", 4), ("r", 3),
                ("s", 6), ("q", 5), ("r", 4),
                ("q", 6), ("r", 5), ("r", 6),
            ]
            u1 = {}
            u2 = {}
            ready = []
            for idx, (kind, t) in enumerate(se_steps):
                if kind == "p":
                    u1[t] = conv1_pack(t)
                elif kind == "s":
                    conv1_sel(t, u1[t])
                elif kind == "q":
                    u2[t] = conv2_pack(t)
                else:
                    conv2_sel(t, u2[t])
                    ready.append((t, 0))
                    ready.append((t, 1))
                if 10 <= idx <= 21 and ready:
                    main_group(*ready.pop(0))
            for tc in ready:
                main_group(*tc)

    nc.compile()
    return nc


_NC = None


def make_in_maps(x, weight, A_w, se_w1, se_w2):
    import ml_dtypes

    bf16 = ml_dtypes.bfloat16
    x = np.asarray(x, dtype=np.float32)
    # pre-padded x: [B, ci-block, 128, guard + 56*58 + guard] with zero pad
    # columns (c=0, c=57) and guards
    xpad = np.zeros((B, 2, 128, HWP + 2), dtype=bf16)
    xv = xpad[:, :, :, 1 : 1 + HWP].reshape(B, 2, 128, H, WP)
    xv[:, :, :, :, 1 : W + 1] = x.reshape(B, 2, 128, H, W).astype(bf16)

    # fold A_w into the conv weight on host (f32), then transpose+cast
    wm = np.asarray(weight, dtype=np.float32) * np.asarray(A_w, dtype=np.float32)
    wmodT = np.ascontiguousarray(
        wm.transpose(1, 2, 3, 0).reshape(2, 128, 9 * 256).astype(bf16)
    )

    # SE weights pre-packed: kw groups at 32-col strides per kh slice
    w1T = np.asarray(se_w1, dtype=np.float32).transpose(1, 2, 3, 0)  # [ci,kh,kw,16]
    w1P = np.zeros((2, 128, 3, 3, 32), dtype=bf16)
    w1P[:, :, :, :, :CMID] = w1T.reshape(2, 128, 3, 3, CMID).astype(bf16)
    w1P = np.ascontiguousarray(w1P.reshape(2, 128, 3 * 96))
    w2P = np.zeros((CMID, 3, 3, 32), dtype=bf16)
    w2P[:, :, :, :CMID] = (
        np.asarray(se_w2, dtype=np.float32)[0].astype(bf16)[:, :, :, None]
    )
    w2P = np.ascontiguousarray(w2P.reshape(CMID, 3 * 96))

    in_maps = [
        {
            "xpad": np.ascontiguousarray(xpad[b]),
            "wmodT": wmodT,
            "se_w1P": w1P,
            "se_w2P": w2P,
        }
        for b in range(B)
    ]
    return in_maps


def kernel(x, weight, A_w, se_w1, se_w2):
    global _NC
    if _NC is None:
        _NC = build()
    in_maps = make_in_maps(x, weight, A_w, se_w1, se_w2)
    res = run_bass_kernel_spmd(_NC, in_maps, list(range(N_CORES)))
    out = np.stack([res.results[b]["outp"] for b in range(B)], axis=0)
    # strip pad columns: [B,2,128,56*58] -> [B,256,56,56]
    out = out.reshape(B, 2, 128, H, WP)[:, :, :, :, 1 : W + 1].reshape(B, C, H, W)
    return np.ascontiguousarray(out)
